# revision 7
# baseline (speedup 1.0000x reference)
"""Trainium2 Bass kernel for single-head dense attention without softmax.

Reference computation (B=4, S=4096, H=1024, fp32):
    q    = x @ W^T               [B, S, H]
    attn = (q @ x^T) @ x         [B, S, H]

There is no softmax, so the computation reorders to
    attn[b] = x[b] @ (W^T @ (x[b]^T @ x[b]))
which drops the FLOP count from ~309 GF to ~77 GF total.

Sharding over 8 NeuronCores: core c handles batch b = c//2 and output
columns jcols = [512*j, 512*j+512) with j = c%2.  Each core computes
    G = x[b]^T x[b]  restricted to columns jcols       (pass 1)
    C = W^T G[:, jcols]                                (pass 2)
    out[:, jcols] = x[b] @ C                           (pass 3)
To keep the device program identical across cores (SPMD), the host
permutes the H columns of x (and the H rows of W) per core so the
core's jcols always land in columns [0, 512).

Precision: pass 1 runs in fp8-e4m3 with DoubleRow perf mode (two
contraction tiles per matmul); passes 2/3 run in bf16.  G/C accumulate
in fp32 PSUM; the output is written in fp32.  Measured rel-err vs the
fp32 reference is ~1.6e-2 (gate: 2e-2); inputs are deterministic so
this margin is stable.  Set P1_FP8=False for an all-bf16 kernel
(~3.9e-3).

Layout/overlap notes (from trace analysis):
  - x / xt / o are pre-tiled on the host into exact per-chunk SBUF
    images so every DMA moves >=4 KiB contiguous per partition
    (fragmented APs measured ~150 GB/s vs ~350+ GB/s contiguous).
  - all 8 xt chunks fit in the stream pool and prefetch during
    passes 1-2, so pass 3 never waits on input DMA.
  - pass 3 accumulates 4 s-tiles concurrently in 4 PSUM banks
    (h outer, s-tile inner): back-to-back matmuls hit different banks
    and reuse the same moving operand, ~145 ns/MM vs ~216 ns/MM for
    same-bank accumulation.
  - outputs go out 2 s-tiles per DMA, alternating the sync/scalar
    HWDGE rings, so the per-DMA fixed cost (~1.3 us) stays off the
    critical path and the end-of-kernel drain is short.
  - PSUM->SBUF copies alternate vector/scalar engines.
"""

import sys
import types

import numpy as np
import ml_dtypes

import concourse.mybir as mybir
import concourse.tile as tile
from concourse import bacc
from concourse.bass_utils import run_bass_kernel_spmd

# bass_utils imports antenv.axon_hooks when tracing is requested (even via a
# stray BASS_TRACE env var); the module is absent in this image, so provide a
# no-op fallback unless someone already registered a real one.
if "antenv.axon_hooks" not in sys.modules:
    try:
        import antenv.axon_hooks  # noqa: F401
    except ImportError:
        _m = types.ModuleType("antenv.axon_hooks")
        _m.get_axon_ntff_profile_hook = lambda: None
        _m.set_axon_ntff_profile_hook = lambda h: None
        sys.modules["antenv.axon_hooks"] = _m

P = 128          # partitions / matmul contraction tile
S = 4096         # sequence length
H = 1024         # hidden
NJ = 512         # output columns per core
KS = S // P      # 32 sequence tiles
KH = H // P      # 8 hidden tiles
N_CORES = 8

BF = mybir.dt.bfloat16
F8 = mybir.dt.float8e4
F32 = mybir.dt.float32

P1_FP8 = True    # pass 1 in fp8-e4m3 DoubleRow (else bf16)

# pass-1 k-tile chunk sizes (one DMA each); small head chunks so the first
# matmul waits on the minimum bytes
CHUNKS = [2, 2, 4, 8, 8, 8]
assert sum(CHUNKS) == KS
SCC = 512                 # xt chunk width in s-columns (1 MiB bf16)
NSC = S // SCC
ST = SCC // P             # s-tiles per xt chunk (4)

NP_F8 = ml_dtypes.float8_e4m3   # TRN FP8_EXP4: e4m3 with inf, max +-240
NP_BF = ml_dtypes.bfloat16

_CACHE: dict = {}


def build_kernel(p1_fp8=P1_FP8):
    nc = bacc.Bacc("TRN2", target_bir_lowering=False, debug=False)

    xdt = F8 if p1_fp8 else BF
    # x pre-tiled into chunk images: chunk c is [P, CHUNKS[c]*H], flat
    x_ext = nc.dram_tensor("x", [KS * P, H], xdt, kind="ExternalInput")
    # xt pre-tiled: [NSC, P, KH, SCC] chunk images
    xt_ext = nc.dram_tensor("xt", [NSC * P, KH * SCC], BF, kind="ExternalInput")
    w_ext = nc.dram_tensor("w", [H, H], BF, kind="ExternalInput")
    # o tiled [NSC*2, P, 2, NJ]: host de-tiles
    o_ext = nc.dram_tensor("o", [NSC * 2 * P, 2 * NJ], F32, kind="ExternalOutput")

    kstep = 2 if p1_fp8 else 1            # k-tiles consumed per matmul
    pm = mybir.MatmulPerfMode.DoubleRow if p1_fp8 else None
    w_r = w_ext.ap().rearrange("(kw p) h -> kw p h", p=P)
    xt_r = xt_ext.ap().rearrange("(sc p) (ho s) -> sc p ho s", p=P, s=SCC)
    o_r = o_ext.ap().rearrange("(r p) (two j) -> r p two j", p=P, j=NJ)

    with tile.TileContext(nc) as tc:
        with (
            tc.tile_pool(name="stream", bufs=10) as stream_pool,
            tc.tile_pool(name="wk", bufs=8) as wk_pool,
            tc.tile_pool(name="gc", bufs=1) as gc_pool,
            tc.tile_pool(name="ot", bufs=6) as ot_pool,
            tc.tile_pool(name="ps", bufs=8, space="PSUM") as ps_pool,
        ):
            # PE warmup: dummy matmuls on a zero tile while the first x DMA
            # is in flight (cold PE runs at 1.2 GHz until ~3.4us of activity)
            warm = gc_pool.tile([P, NJ + P], BF, name="warm")
            nc.vector.memset(warm[:, 0:8], 0.0)
            warm_ps = ps_pool.tile([P, NJ], F32, tag="ps", name="warm_ps")
            for _ in range(3):
                nc.tensor.matmul(
                    warm_ps[:], warm[:, 0:P], warm[:, P : P + NJ], start=True, stop=True
                )

            # ---- pass 1: G[:, 0:512] = (x^T x)[:, 0:512] ----
            g_sb = [gc_pool.tile([P, NJ], BF, name=f"g_sb{i}") for i in range(KH)]
            g_ps = [ps_pool.tile([P, NJ], F32, tag="ps", name=f"g_ps{i}") for i in range(KH)]
            wks = []
            xt_pre = []
            kt0 = 0
            for ci, nk in enumerate(CHUNKS):
                tag = "head" if ci == 0 else "stream"
                xs = stream_pool.tile([P, nk, H], xdt, tag=tag, bufs=1 if ci == 0 else None,
                                      name=f"xs{ci}")
                src = (
                    x_ext.ap()[kt0 * P : (kt0 + nk) * P, :]
                    .rearrange("(kt p) h -> p kt h", p=P)
                )
                nc.sync.dma_start(xs[:], src)
                for ki in range(0, nk, kstep):
                    for mi in range(KH):
                        if p1_fp8:
                            lhsT = xs[:, ki : ki + kstep, mi * P : (mi + 1) * P]
                            rhs = xs[:, ki : ki + kstep, 0:NJ]
                        else:
                            lhsT = xs[:, ki, mi * P : (mi + 1) * P]
                            rhs = xs[:, ki, 0:NJ]
                        nc.tensor.matmul(
                            g_ps[mi][:],
                            lhsT,
                            rhs,
                            start=(kt0 == 0 and ki == 0),
                            stop=(ci == len(CHUNKS) - 1 and ki == nk - kstep),
                            perf_mode=pm,
                        )
                kt0 += nk
                # W prefetch through the back half of pass 1
                if ci >= len(CHUNKS) - 2:
                    kw0 = (ci - (len(CHUNKS) - 2)) * 4
                    for kw in range(kw0, kw0 + 4):
                        wk = wk_pool.tile([P, H], BF, tag="wk", name=f"wk{kw}")
                        nc.sync.dma_start(wk[:], w_r[kw])
                        wks.append(wk)
            # prefetch ALL xt chunks; they reuse stream slots as x retires
            for sc in range(NSC):
                xt_c = stream_pool.tile([P, KH, SCC], BF, tag="stream", name=f"xt{sc}")
                nc.sync.dma_start(xt_c[:], xt_r[sc])
                xt_pre.append(xt_c)
            for mi in range(KH):
                if mi % 2 == 0:
                    nc.vector.tensor_copy(g_sb[mi][:], g_ps[mi][:])
                else:
                    nc.scalar.copy(g_sb[mi][:], g_ps[mi][:])

            # ---- pass 2: C = W^T G ----
            c_sb = gc_pool.tile([P, KH, NJ], BF)
            c_ps = [ps_pool.tile([P, NJ], F32, tag="ps", name=f"c_ps{i}") for i in range(KH)]
            for k2 in range(KH):
                for hi in range(KH):
                    nc.tensor.matmul(
                        c_ps[hi][:],
                        wks[k2][:, hi * P : (hi + 1) * P],
                        g_sb[k2][:],
                        start=(k2 == 0),
                        stop=(k2 == KH - 1),
                    )
            for hi in range(KH):
                if hi % 2 == 0:
                    nc.vector.tensor_copy(c_sb[:, hi, :], c_ps[hi][:])
                else:
                    nc.scalar.copy(c_sb[:, hi, :], c_ps[hi][:])

            # ---- pass 3: out = x @ C  (x supplied transposed) ----
            # 4 s-tiles accumulate concurrently in 4 PSUM banks; the moving
            # operand c_sb[:, h, :] is reused across the inner s loop
            for sc in range(NSC):
                xt_c = xt_pre[sc]
                o_ps = [ps_pool.tile([P, NJ], F32, tag="ps", name=f"o_ps{sc}_{i}") for i in range(ST)]
                for h in range(KH):
                    for ss in range(ST):
                        nc.tensor.matmul(
                            o_ps[ss][:],
                            xt_c[:, h, ss * P : (ss + 1) * P],
                            c_sb[:, h, :],
                            start=(h == 0),
                            stop=(h == KH - 1),
                        )
                # copy out 2 s-tiles per SBUF tile / DMA, alternating engines
                for half in range(ST // 2):
                    o_t = ot_pool.tile([P, 2, NJ], F32, tag="ot", name=f"o_t{sc}_{half}")
                    for i in range(2):
                        if half == 0:
                            nc.vector.tensor_copy(o_t[:, i, :], o_ps[half * 2 + i][:])
                        else:
                            nc.scalar.copy(o_t[:, i, :], o_ps[half * 2 + i][:])
                    deng = nc.scalar if (sc * 2 + half) % 2 == 0 else nc.sync
                    deng.dma_start(o_r[sc * 2 + half], o_t[:])

    nc.compile()
    return nc


def make_in_maps(hidden_states: np.ndarray, W_q: np.ndarray, p1_fp8=P1_FP8):
    """Shard full inputs into the 8 per-core input maps (pre-tiled)."""
    x = np.asarray(hidden_states, dtype=np.float32)
    w = np.asarray(W_q, dtype=np.float32)
    np_xdt = NP_F8 if p1_fp8 else NP_BF
    perms = [np.arange(H), np.r_[H // 2 : H, 0 : H // 2]]
    in_maps = []
    for c in range(N_CORES):
        b, j = c // 2, c % 2
        xb = x[b]
        xp = xb[:, perms[j]]
        # chunk images: for chunk (kt0, nk): [P, nk, H] with (p, kt, h) =
        # x[(kt0+kt)*P + p, h]; concatenated flat as [KS*P, H]
        parts = []
        kt0 = 0
        for nk in CHUNKS:
            blk = xp[kt0 * P : (kt0 + nk) * P, :].reshape(nk, P, H)
            parts.append(blk.transpose(1, 0, 2).reshape(P, nk * H))
            kt0 += nk
        x_in = np.concatenate([p.reshape(-1, H) for p in parts], axis=0)
        # xt chunks [NSC, P, KH, SCC]: (p, ho, s) = x[sc*SCC+s, ho*P+p]
        xtt = (
            xb.reshape(NSC, SCC, KH, P).transpose(0, 3, 2, 1).reshape(NSC * P, KH * SCC)
        )
        in_maps.append(
            {
                "x": np.ascontiguousarray(x_in).astype(np_xdt),
                "xt": np.ascontiguousarray(xtt).astype(NP_BF),
                "w": np.ascontiguousarray(w[perms[j], :]).astype(NP_BF),
            }
        )
    return in_maps


def untile_out(o_res: np.ndarray) -> np.ndarray:
    """[NSC*2*P, 2*NJ] tiled output -> [S, NJ]."""
    return (
        o_res.reshape(NSC * 2, P, 2, NJ).transpose(0, 2, 1, 3).reshape(S, NJ)
    )


def run(hidden_states: np.ndarray, W_q: np.ndarray, **run_kwargs):
    """Build (cached), run on 8 cores, gather.  Returns (output, results)."""
    if "nc" not in _CACHE:
        _CACHE["nc"] = build_kernel()
    nc = _CACHE["nc"]
    in_maps = make_in_maps(hidden_states, W_q)
    res = run_bass_kernel_spmd(nc, in_maps, list(range(N_CORES)), **run_kwargs)
    B = N_CORES // 2
    out = np.empty((B, S, H), dtype=np.float32)
    for c in range(N_CORES):
        b, j = c // 2, c % 2
        out[b, :, j * NJ : (j + 1) * NJ] = untile_out(res.results[c]["o"])
    return out, res


def kernel(hidden_states: np.ndarray, W_q: np.ndarray, **unused) -> np.ndarray:
    out, _ = run(hidden_states, W_q)
    return out


if __name__ == "__main__":
    rng = np.random.default_rng(0)
    x = rng.standard_normal((4, S, H), dtype=np.float32)
    w = (rng.standard_normal((H, H), dtype=np.float32) * 9.02e-5).astype(np.float32)
    out = kernel(hidden_states=x, W_q=w)
    xb = x[0].astype(np.float64)
    ref0 = xb @ w.astype(np.float64).T @ (xb.T @ xb)
    err = np.abs(out[0] - ref0) / (np.abs(ref0).max() + 1e-30)
    print("max scale-relative err (batch 0):", err.max())


# revision 8
# speedup vs baseline: 1.0002x; 1.0002x over previous
"""Trainium2 Bass kernel for single-head dense attention without softmax.

Reference computation (B=4, S=4096, H=1024, fp32):
    q    = x @ W^T               [B, S, H]
    attn = (q @ x^T) @ x         [B, S, H]

There is no softmax, so the computation reorders to
    attn[b] = x[b] @ (W^T @ (x[b]^T @ x[b]))
which drops the FLOP count from ~309 GF to ~77 GF total.

Sharding over 8 NeuronCores: core c handles batch b = c//2 and output
columns jcols = [512*j, 512*j+512) with j = c%2.  Each core computes
    G = x[b]^T x[b]  restricted to columns jcols       (pass 1)
    C = W^T G[:, jcols]                                (pass 2)
    out[:, jcols] = x[b] @ C                           (pass 3)
To keep the device program identical across cores (SPMD), the host
permutes the H columns of x (and the H rows of W) per core so the
core's jcols always land in columns [0, 512).

Precision: pass 1 runs in fp8-e4m3 with DoubleRow perf mode (two
contraction tiles per matmul); passes 2/3 run in bf16.  G/C accumulate
in fp32 PSUM; the output is written in fp32.  Measured rel-err vs the
fp32 reference is ~1.6e-2 (gate: 2e-2); inputs are deterministic so
this margin is stable.  Set P1_FP8=False for an all-bf16 kernel
(~3.9e-3).

Layout/overlap notes (from trace analysis):
  - x / xt / o are pre-tiled on the host into exact per-chunk SBUF
    images so every DMA moves >=4 KiB contiguous per partition
    (fragmented APs measured ~150 GB/s vs ~350+ GB/s contiguous).
  - all 8 xt chunks fit in the stream pool and prefetch during
    passes 1-2, so pass 3 never waits on input DMA.
  - pass 3 accumulates 4 s-tiles concurrently in 4 PSUM banks
    (h outer, s-tile inner): back-to-back matmuls hit different banks
    and reuse the same moving operand, ~145 ns/MM vs ~216 ns/MM for
    same-bank accumulation.
  - outputs go out 2 s-tiles per DMA, alternating the sync/scalar
    HWDGE rings, so the per-DMA fixed cost (~1.3 us) stays off the
    critical path and the end-of-kernel drain is short.
  - PSUM->SBUF copies alternate vector/scalar engines.
"""

import sys
import types

import numpy as np
import ml_dtypes

import concourse.mybir as mybir
import concourse.tile as tile
from concourse import bacc
from concourse.bass_utils import run_bass_kernel_spmd

# bass_utils imports antenv.axon_hooks when tracing is requested (even via a
# stray BASS_TRACE env var); the module is absent in this image, so provide a
# no-op fallback unless someone already registered a real one.
if "antenv.axon_hooks" not in sys.modules:
    try:
        import antenv.axon_hooks  # noqa: F401
    except ImportError:
        _m = types.ModuleType("antenv.axon_hooks")
        _m.get_axon_ntff_profile_hook = lambda: None
        _m.set_axon_ntff_profile_hook = lambda h: None
        sys.modules["antenv.axon_hooks"] = _m

P = 128          # partitions / matmul contraction tile
S = 4096         # sequence length
H = 1024         # hidden
NJ = 512         # output columns per core
KS = S // P      # 32 sequence tiles
KH = H // P      # 8 hidden tiles
N_CORES = 8

BF = mybir.dt.bfloat16
F8 = mybir.dt.float8e4
F32 = mybir.dt.float32

P1_FP8 = True    # pass 1 in fp8-e4m3 DoubleRow (else bf16)

# pass-1 k-tile chunk sizes (one DMA each); small head chunks so the first
# matmul waits on the minimum bytes
CHUNKS = [4, 4, 8, 8, 8]
assert sum(CHUNKS) == KS
SCC = 512                 # xt chunk width in s-columns (1 MiB bf16)
NSC = S // SCC
ST = SCC // P             # s-tiles per xt chunk (4)

NP_F8 = ml_dtypes.float8_e4m3   # TRN FP8_EXP4: e4m3 with inf, max +-240
NP_BF = ml_dtypes.bfloat16

_CACHE: dict = {}


def build_kernel(p1_fp8=P1_FP8):
    nc = bacc.Bacc("TRN2", target_bir_lowering=False, debug=False)

    xdt = F8 if p1_fp8 else BF
    # x pre-tiled into chunk images: chunk c is [P, CHUNKS[c]*H], flat
    x_ext = nc.dram_tensor("x", [KS * P, H], xdt, kind="ExternalInput")
    # xt pre-tiled: [NSC, P, KH, SCC] chunk images
    xt_ext = nc.dram_tensor("xt", [NSC * P, KH * SCC], BF, kind="ExternalInput")
    w_ext = nc.dram_tensor("w", [H, H], BF, kind="ExternalInput")
    # o tiled [NSC*2, P, 2, NJ]: host de-tiles
    o_ext = nc.dram_tensor("o", [NSC * 2 * P, 2 * NJ], F32, kind="ExternalOutput")

    kstep = 2 if p1_fp8 else 1            # k-tiles consumed per matmul
    pm = mybir.MatmulPerfMode.DoubleRow if p1_fp8 else None
    w_r = w_ext.ap().rearrange("(kw p) h -> kw p h", p=P)
    xt_r = xt_ext.ap().rearrange("(sc p) (ho s) -> sc p ho s", p=P, s=SCC)
    o_r = o_ext.ap().rearrange("(r p) (two j) -> r p two j", p=P, j=NJ)

    with tile.TileContext(nc) as tc:
        with (
            tc.tile_pool(name="stream", bufs=10) as stream_pool,
            tc.tile_pool(name="wk", bufs=8) as wk_pool,
            tc.tile_pool(name="gc", bufs=1) as gc_pool,
            tc.tile_pool(name="ot", bufs=6) as ot_pool,
            tc.tile_pool(name="ps", bufs=8, space="PSUM") as ps_pool,
        ):
            # PE warmup: dummy matmuls on a zero tile while the first x DMA
            # is in flight (cold PE runs at 1.2 GHz until ~3.4us of activity)
            warm = gc_pool.tile([P, NJ + P], BF, name="warm")
            nc.vector.memset(warm[:, 0:8], 0.0)
            # dummy DMA to absorb the DMA-queue init latency before the
            # first real chunk lands on the same ring
            nc.sync.dma_start(warm[:, NJ + 8 : NJ + 16], w_ext.ap()[0:P, 0:8])
            warm_ps = ps_pool.tile([P, NJ], F32, tag="ps", name="warm_ps")
            for _ in range(8):
                nc.tensor.matmul(
                    warm_ps[:], warm[:, 0:P], warm[:, P : P + NJ], start=True, stop=True
                )

            # ---- pass 1: G[:, 0:512] = (x^T x)[:, 0:512] ----
            g_sb = [gc_pool.tile([P, NJ], BF, name=f"g_sb{i}") for i in range(KH)]
            g_ps = [ps_pool.tile([P, NJ], F32, tag="ps", name=f"g_ps{i}") for i in range(KH)]
            wks = []
            xt_pre = []
            kt0 = 0
            for ci, nk in enumerate(CHUNKS):
                tag = "head" if ci == 0 else "stream"
                xs = stream_pool.tile([P, nk, H], xdt, tag=tag, bufs=1 if ci == 0 else None,
                                      name=f"xs{ci}")
                src = (
                    x_ext.ap()[kt0 * P : (kt0 + nk) * P, :]
                    .rearrange("(kt p) h -> p kt h", p=P)
                )
                nc.sync.dma_start(xs[:], src)
                for ki in range(0, nk, kstep):
                    for mi in range(KH):
                        if p1_fp8:
                            lhsT = xs[:, ki : ki + kstep, mi * P : (mi + 1) * P]
                            rhs = xs[:, ki : ki + kstep, 0:NJ]
                        else:
                            lhsT = xs[:, ki, mi * P : (mi + 1) * P]
                            rhs = xs[:, ki, 0:NJ]
                        nc.tensor.matmul(
                            g_ps[mi][:],
                            lhsT,
                            rhs,
                            start=(kt0 == 0 and ki == 0),
                            stop=(ci == len(CHUNKS) - 1 and ki == nk - kstep),
                            perf_mode=pm,
                        )
                kt0 += nk
                # W prefetch through the back half of pass 1
                if ci >= len(CHUNKS) - 2:
                    kw0 = (ci - (len(CHUNKS) - 2)) * 4
                    for kw in range(kw0, kw0 + 4):
                        wk = wk_pool.tile([P, H], BF, tag="wk", name=f"wk{kw}")
                        nc.sync.dma_start(wk[:], w_r[kw])
                        wks.append(wk)
            # prefetch ALL xt chunks; they reuse stream slots as x retires
            for sc in range(NSC):
                xt_c = stream_pool.tile([P, KH, SCC], BF, tag="stream", name=f"xt{sc}")
                nc.sync.dma_start(xt_c[:], xt_r[sc])
                xt_pre.append(xt_c)
            for mi in range(KH):
                if mi % 2 == 0:
                    nc.vector.tensor_copy(g_sb[mi][:], g_ps[mi][:])
                else:
                    nc.scalar.copy(g_sb[mi][:], g_ps[mi][:])

            # ---- pass 2: C = W^T G ----
            c_sb = gc_pool.tile([P, KH, NJ], BF)
            c_ps = [ps_pool.tile([P, NJ], F32, tag="ps", name=f"c_ps{i}") for i in range(KH)]
            for k2 in range(KH):
                for hi in range(KH):
                    nc.tensor.matmul(
                        c_ps[hi][:],
                        wks[k2][:, hi * P : (hi + 1) * P],
                        g_sb[k2][:],
                        start=(k2 == 0),
                        stop=(k2 == KH - 1),
                    )
            for hi in range(KH):
                if hi % 2 == 0:
                    nc.vector.tensor_copy(c_sb[:, hi, :], c_ps[hi][:])
                else:
                    nc.scalar.copy(c_sb[:, hi, :], c_ps[hi][:])

            # ---- pass 3: out = x @ C  (x supplied transposed) ----
            # 4 s-tiles accumulate concurrently in 4 PSUM banks; the moving
            # operand c_sb[:, h, :] is reused across the inner s loop
            for sc in range(NSC):
                xt_c = xt_pre[sc]
                o_ps = [ps_pool.tile([P, NJ], F32, tag="ps", name=f"o_ps{sc}_{i}") for i in range(ST)]
                for h in range(KH):
                    for ss in range(ST):
                        nc.tensor.matmul(
                            o_ps[ss][:],
                            xt_c[:, h, ss * P : (ss + 1) * P],
                            c_sb[:, h, :],
                            start=(h == 0),
                            stop=(h == KH - 1),
                        )
                if sc < NSC - 1:
                    # copy out 2 s-tiles per SBUF tile / DMA, alternating
                    # engines and HWDGE rings
                    for half in range(ST // 2):
                        o_t = ot_pool.tile([P, 2, NJ], F32, tag="ot", name=f"o_t{sc}_{half}")
                        for i in range(2):
                            if half == 0:
                                nc.vector.tensor_copy(o_t[:, i, :], o_ps[half * 2 + i][:])
                            else:
                                nc.scalar.copy(o_t[:, i, :], o_ps[half * 2 + i][:])
                        deng = nc.scalar if (sc * 2 + half) % 2 == 0 else nc.sync
                        deng.dma_start(o_r[sc * 2 + half], o_t[:])
                else:
                    # last chunk: per-tile copies/DMAs so the drain after the
                    # final matmul is as short as possible
                    for ss in range(ST):
                        o_t = ot_pool.tile([P, 1, NJ], F32, tag="ot1", bufs=4,
                                           name=f"o_tl{ss}")
                        if ss % 2 == 0:
                            nc.vector.tensor_copy(o_t[:, 0, :], o_ps[ss][:])
                        else:
                            nc.scalar.copy(o_t[:, 0, :], o_ps[ss][:])
                        deng = nc.scalar if ss % 2 == 0 else nc.sync
                        deng.dma_start(o_r[sc * 2 + ss // 2, :, ss % 2, :], o_t[:, 0, :])

    nc.compile()
    return nc


def make_in_maps(hidden_states: np.ndarray, W_q: np.ndarray, p1_fp8=P1_FP8):
    """Shard full inputs into the 8 per-core input maps (pre-tiled)."""
    x = np.asarray(hidden_states, dtype=np.float32)
    w = np.asarray(W_q, dtype=np.float32)
    np_xdt = NP_F8 if p1_fp8 else NP_BF
    perms = [np.arange(H), np.r_[H // 2 : H, 0 : H // 2]]
    in_maps = []
    for c in range(N_CORES):
        b, j = c // 2, c % 2
        xb = x[b]
        xp = xb[:, perms[j]]
        # chunk images: for chunk (kt0, nk): [P, nk, H] with (p, kt, h) =
        # x[(kt0+kt)*P + p, h]; concatenated flat as [KS*P, H]
        parts = []
        kt0 = 0
        for nk in CHUNKS:
            blk = xp[kt0 * P : (kt0 + nk) * P, :].reshape(nk, P, H)
            parts.append(blk.transpose(1, 0, 2).reshape(P, nk * H))
            kt0 += nk
        x_in = np.concatenate([p.reshape(-1, H) for p in parts], axis=0)
        # xt chunks [NSC, P, KH, SCC]: (p, ho, s) = x[sc*SCC+s, ho*P+p]
        xtt = (
            xb.reshape(NSC, SCC, KH, P).transpose(0, 3, 2, 1).reshape(NSC * P, KH * SCC)
        )
        in_maps.append(
            {
                "x": np.ascontiguousarray(x_in).astype(np_xdt),
                "xt": np.ascontiguousarray(xtt).astype(NP_BF),
                "w": np.ascontiguousarray(w[perms[j], :]).astype(NP_BF),
            }
        )
    return in_maps


def untile_out(o_res: np.ndarray) -> np.ndarray:
    """[NSC*2*P, 2*NJ] tiled output -> [S, NJ]."""
    return (
        o_res.reshape(NSC * 2, P, 2, NJ).transpose(0, 2, 1, 3).reshape(S, NJ)
    )


def run(hidden_states: np.ndarray, W_q: np.ndarray, **run_kwargs):
    """Build (cached), run on 8 cores, gather.  Returns (output, results)."""
    if "nc" not in _CACHE:
        _CACHE["nc"] = build_kernel()
    nc = _CACHE["nc"]
    in_maps = make_in_maps(hidden_states, W_q)
    res = run_bass_kernel_spmd(nc, in_maps, list(range(N_CORES)), **run_kwargs)
    B = N_CORES // 2
    out = np.empty((B, S, H), dtype=np.float32)
    for c in range(N_CORES):
        b, j = c // 2, c % 2
        out[b, :, j * NJ : (j + 1) * NJ] = untile_out(res.results[c]["o"])
    return out, res


def kernel(hidden_states: np.ndarray, W_q: np.ndarray, **unused) -> np.ndarray:
    out, _ = run(hidden_states, W_q)
    return out


if __name__ == "__main__":
    rng = np.random.default_rng(0)
    x = rng.standard_normal((4, S, H), dtype=np.float32)
    w = (rng.standard_normal((H, H), dtype=np.float32) * 9.02e-5).astype(np.float32)
    out = kernel(hidden_states=x, W_q=w)
    xb = x[0].astype(np.float64)
    ref0 = xb @ w.astype(np.float64).T @ (xb.T @ xb)
    err = np.abs(out[0] - ref0) / (np.abs(ref0).max() + 1e-30)
    print("max scale-relative err (batch 0):", err.max())


# revision 10
# speedup vs baseline: 1.0228x; 1.0226x over previous
"""Trainium2 Bass kernel for single-head dense attention without softmax.

Reference computation (B=4, S=4096, H=1024, fp32):
    q    = x @ W^T               [B, S, H]
    attn = (q @ x^T) @ x         [B, S, H]

There is no softmax, so the computation reorders to
    attn[b] = x[b] @ (W^T @ (x[b]^T @ x[b]))
which drops the FLOP count from ~309 GF to ~77 GF total.

Sharding over 8 NeuronCores: core c handles batch b = c//2 and output
columns jcols = [512*j, 512*j+512) with j = c%2.  Each core computes
    G = x[b]^T x[b]  restricted to columns jcols       (pass 1)
    C = W^T G[:, jcols]                                (pass 2)
    out[:, jcols] = x[b] @ C                           (pass 3)
To keep the device program identical across cores (SPMD), the host
permutes the H columns of x (and the H rows of W) per core so the
core's jcols always land in columns [0, 512).

Precision: pass 1 runs in fp8-e4m3 with DoubleRow perf mode (two
contraction tiles per matmul); passes 2/3 run in bf16.  G/C accumulate
in fp32 PSUM; the output is written in fp32.  Measured rel-err vs the
fp32 reference is ~1.6e-2 (gate: 2e-2); inputs are deterministic so
this margin is stable.  Set P1_FP8=False for an all-bf16 kernel
(~3.9e-3).

Layout/overlap notes (from trace analysis):
  - x / xt / o are pre-tiled on the host into exact per-chunk SBUF
    images so every DMA moves >=4 KiB contiguous per partition
    (fragmented APs measured ~150 GB/s vs ~350+ GB/s contiguous).
  - all 8 xt chunks fit in the stream pool and prefetch during
    passes 1-2, so pass 3 never waits on input DMA.
  - pass 3 accumulates 4 s-tiles concurrently in 4 PSUM banks
    (h outer, s-tile inner): back-to-back matmuls hit different banks
    and reuse the same moving operand, ~145 ns/MM vs ~216 ns/MM for
    same-bank accumulation.
  - outputs go out 2 s-tiles per DMA, alternating the sync/scalar
    HWDGE rings, so the per-DMA fixed cost (~1.3 us) stays off the
    critical path and the end-of-kernel drain is short.
  - PSUM->SBUF copies alternate vector/scalar engines.
"""

import sys
import types

import numpy as np
import ml_dtypes

import concourse.mybir as mybir
import concourse.tile as tile
from concourse import bacc
from concourse.bass_utils import run_bass_kernel_spmd

# bass_utils imports antenv.axon_hooks when tracing is requested (even via a
# stray BASS_TRACE env var); the module is absent in this image, so provide a
# no-op fallback unless someone already registered a real one.
if "antenv.axon_hooks" not in sys.modules:
    try:
        import antenv.axon_hooks  # noqa: F401
    except ImportError:
        _m = types.ModuleType("antenv.axon_hooks")
        _m.get_axon_ntff_profile_hook = lambda: None
        _m.set_axon_ntff_profile_hook = lambda h: None
        sys.modules["antenv.axon_hooks"] = _m

P = 128          # partitions / matmul contraction tile
S = 4096         # sequence length
H = 1024         # hidden
NJ = 512         # output columns per core
KS = S // P      # 32 sequence tiles
KH = H // P      # 8 hidden tiles
N_CORES = 8

BF = mybir.dt.bfloat16
F8 = mybir.dt.float8e4
F32 = mybir.dt.float32

P1_FP8 = True    # pass 1 in fp8-e4m3 DoubleRow (else bf16)

# pass-1 k-tile chunk sizes (one DMA each); small head chunks so the first
# matmul waits on the minimum bytes
CHUNKS = [4, 4, 8, 8, 8]
assert sum(CHUNKS) == KS
SCC = 512                 # xt chunk width in s-columns (1 MiB bf16)
NSC = S // SCC
ST = SCC // P             # s-tiles per xt chunk (4)

NP_F8 = ml_dtypes.float8_e4m3   # TRN FP8_EXP4: e4m3 with inf, max +-240
NP_BF = ml_dtypes.bfloat16

_CACHE: dict = {}


def build_kernel(p1_fp8=P1_FP8):
    nc = bacc.Bacc("TRN2", target_bir_lowering=False, debug=False)

    xdt = F8 if p1_fp8 else BF
    # x pre-tiled into chunk images: chunk c is [P, CHUNKS[c]*H], flat
    x_ext = nc.dram_tensor("x", [KS * P, H], xdt, kind="ExternalInput")
    # xt pre-tiled: [NSC, P, KH, SCC] chunk images
    xt_ext = nc.dram_tensor("xt", [NSC * P, KH * SCC], BF, kind="ExternalInput")
    w_ext = nc.dram_tensor("w", [H, H], BF, kind="ExternalInput")
    # o tiled [NSC*2, P, 2, NJ]: host de-tiles
    o_ext = nc.dram_tensor("o", [NSC * 2 * P, 2 * NJ], F32, kind="ExternalOutput")

    kstep = 2 if p1_fp8 else 1            # k-tiles consumed per matmul
    pm = mybir.MatmulPerfMode.DoubleRow if p1_fp8 else None
    w_r = w_ext.ap().rearrange("(kw p) h -> kw p h", p=P)
    xt_r = xt_ext.ap().rearrange("(sc p) (ho s) -> sc p ho s", p=P, s=SCC)
    o_r = o_ext.ap().rearrange("(r p) (two j) -> r p two j", p=P, j=NJ)

    import contextlib as _ctxlib
    with tile.TileContext(nc) as tc, _ctxlib.ExitStack() as _stk:
        # raw (non-tile) SBUF scratch for PE warmup: contents are garbage and
        # never depended on, so the warm matmuls have no upstream producer
        warm_t = _stk.enter_context(nc.sbuf_tensor([P, NJ + P], BF))
        with (
            tc.tile_pool(name="stream", bufs=10) as stream_pool,
            tc.tile_pool(name="wk", bufs=8) as wk_pool,
            tc.tile_pool(name="gc", bufs=1) as gc_pool,
            tc.tile_pool(name="ot", bufs=6) as ot_pool,
            tc.tile_pool(name="ps", bufs=8, space="PSUM") as ps_pool,
        ):
            # PE warmup: dummy matmuls on a zero tile while the first x DMA
            # is in flight (cold PE runs at 1.2 GHz until ~3.4us of activity)
            warm = warm_t.ap()
            warm_ps = ps_pool.tile([P, NJ], F32, tag="ps", name="warm_ps")
            for _ in range(8):
                nc.tensor.matmul(
                    warm_ps[:], warm[:, 0:P], warm[:, P : P + NJ], start=True, stop=True
                )

            # ---- pass 1: G[:, 0:512] = (x^T x)[:, 0:512] ----
            g_sb = [gc_pool.tile([P, NJ], BF, name=f"g_sb{i}") for i in range(KH)]
            g_ps = [ps_pool.tile([P, NJ], F32, tag="ps", name=f"g_ps{i}") for i in range(KH)]
            wks = []
            xt_pre = []
            kt0 = 0
            for ci, nk in enumerate(CHUNKS):
                tag = "head" if ci == 0 else "stream"
                xs = stream_pool.tile([P, nk, H], xdt, tag=tag, bufs=1 if ci == 0 else None,
                                      name=f"xs{ci}")
                src = (
                    x_ext.ap()[kt0 * P : (kt0 + nk) * P, :]
                    .rearrange("(kt p) h -> p kt h", p=P)
                )
                nc.sync.dma_start(xs[:], src)
                for ki in range(0, nk, kstep):
                    for mi in range(KH):
                        if p1_fp8:
                            lhsT = xs[:, ki : ki + kstep, mi * P : (mi + 1) * P]
                            rhs = xs[:, ki : ki + kstep, 0:NJ]
                        else:
                            lhsT = xs[:, ki, mi * P : (mi + 1) * P]
                            rhs = xs[:, ki, 0:NJ]
                        nc.tensor.matmul(
                            g_ps[mi][:],
                            lhsT,
                            rhs,
                            start=(kt0 == 0 and ki == 0),
                            stop=(ci == len(CHUNKS) - 1 and ki == nk - kstep),
                            perf_mode=pm,
                        )
                kt0 += nk
                # W prefetch through the back half of pass 1
                if ci >= len(CHUNKS) - 2:
                    kw0 = (ci - (len(CHUNKS) - 2)) * 4
                    for kw in range(kw0, kw0 + 4):
                        wk = wk_pool.tile([P, H], BF, tag="wk", name=f"wk{kw}")
                        nc.sync.dma_start(wk[:], w_r[kw])
                        wks.append(wk)
            # prefetch ALL xt chunks; they reuse stream slots as x retires
            for sc in range(NSC):
                xt_c = stream_pool.tile([P, KH, SCC], BF, tag="stream", name=f"xt{sc}")
                nc.sync.dma_start(xt_c[:], xt_r[sc])
                xt_pre.append(xt_c)
            for mi in range(KH):
                if mi % 2 == 0:
                    nc.vector.tensor_copy(g_sb[mi][:], g_ps[mi][:])
                else:
                    nc.scalar.copy(g_sb[mi][:], g_ps[mi][:])

            # ---- pass 2: C = W^T G ----
            c_sb = gc_pool.tile([P, KH, NJ], BF)
            c_ps = [ps_pool.tile([P, NJ], F32, tag="ps", name=f"c_ps{i}") for i in range(KH)]
            for k2 in range(KH):
                for hi in range(KH):
                    nc.tensor.matmul(
                        c_ps[hi][:],
                        wks[k2][:, hi * P : (hi + 1) * P],
                        g_sb[k2][:],
                        start=(k2 == 0),
                        stop=(k2 == KH - 1),
                    )
            for hi in range(KH):
                if hi % 2 == 0:
                    nc.vector.tensor_copy(c_sb[:, hi, :], c_ps[hi][:])
                else:
                    nc.scalar.copy(c_sb[:, hi, :], c_ps[hi][:])

            # ---- pass 3: out = x @ C  (x supplied transposed) ----
            # 4 s-tiles accumulate concurrently in 4 PSUM banks; the moving
            # operand c_sb[:, h, :] is reused across the inner s loop
            for sc in range(NSC):
                xt_c = xt_pre[sc]
                o_ps = [ps_pool.tile([P, NJ], F32, tag="ps", name=f"o_ps{sc}_{i}") for i in range(ST)]
                if sc < NSC - 1:
                    for h in range(KH):
                        for ss in range(ST):
                            nc.tensor.matmul(
                                o_ps[ss][:],
                                xt_c[:, h, ss * P : (ss + 1) * P],
                                c_sb[:, h, :],
                                start=(h == 0),
                                stop=(h == KH - 1),
                            )
                else:
                    # last chunk: finish one s-tile at a time so its copy and
                    # output DMA overlap the remaining matmuls
                    for ss in range(ST):
                        for h in range(KH):
                            nc.tensor.matmul(
                                o_ps[ss][:],
                                xt_c[:, h, ss * P : (ss + 1) * P],
                                c_sb[:, h, :],
                                start=(h == 0),
                                stop=(h == KH - 1),
                            )
                if sc < NSC - 1:
                    # copy out 2 s-tiles per SBUF tile / DMA, alternating
                    # engines and HWDGE rings
                    for half in range(ST // 2):
                        o_t = ot_pool.tile([P, 2, NJ], F32, tag="ot", name=f"o_t{sc}_{half}")
                        for i in range(2):
                            if half == 0:
                                nc.vector.tensor_copy(o_t[:, i, :], o_ps[half * 2 + i][:])
                            else:
                                nc.scalar.copy(o_t[:, i, :], o_ps[half * 2 + i][:])
                        deng = nc.scalar if (sc * 2 + half) % 2 == 0 else nc.sync
                        deng.dma_start(o_r[sc * 2 + half], o_t[:])
                else:
                    # last chunk: per-tile copies/DMAs so the drain after the
                    # final matmul is as short as possible
                    for ss in range(ST):
                        o_t = ot_pool.tile([P, 1, NJ], F32, tag="ot1", bufs=4,
                                           name=f"o_tl{ss}")
                        if ss % 2 == 0:
                            nc.vector.tensor_copy(o_t[:, 0, :], o_ps[ss][:])
                        else:
                            nc.scalar.copy(o_t[:, 0, :], o_ps[ss][:])
                        deng = nc.scalar if ss % 2 == 0 else nc.sync
                        deng.dma_start(o_r[sc * 2 + ss // 2, :, ss % 2, :], o_t[:, 0, :])

    nc.compile()
    return nc


def make_in_maps(hidden_states: np.ndarray, W_q: np.ndarray, p1_fp8=P1_FP8):
    """Shard full inputs into the 8 per-core input maps (pre-tiled)."""
    x = np.asarray(hidden_states, dtype=np.float32)
    w = np.asarray(W_q, dtype=np.float32)
    np_xdt = NP_F8 if p1_fp8 else NP_BF
    perms = [np.arange(H), np.r_[H // 2 : H, 0 : H // 2]]
    in_maps = []
    for c in range(N_CORES):
        b, j = c // 2, c % 2
        xb = x[b]
        xp = xb[:, perms[j]]
        # chunk images: for chunk (kt0, nk): [P, nk, H] with (p, kt, h) =
        # x[(kt0+kt)*P + p, h]; concatenated flat as [KS*P, H]
        parts = []
        kt0 = 0
        for nk in CHUNKS:
            blk = xp[kt0 * P : (kt0 + nk) * P, :].reshape(nk, P, H)
            parts.append(blk.transpose(1, 0, 2).reshape(P, nk * H))
            kt0 += nk
        x_in = np.concatenate([p.reshape(-1, H) for p in parts], axis=0)
        # xt chunks [NSC, P, KH, SCC]: (p, ho, s) = x[sc*SCC+s, ho*P+p]
        xtt = (
            xb.reshape(NSC, SCC, KH, P).transpose(0, 3, 2, 1).reshape(NSC * P, KH * SCC)
        )
        in_maps.append(
            {
                "x": np.ascontiguousarray(x_in).astype(np_xdt),
                "xt": np.ascontiguousarray(xtt).astype(NP_BF),
                "w": np.ascontiguousarray(w[perms[j], :]).astype(NP_BF),
            }
        )
    return in_maps


def untile_out(o_res: np.ndarray) -> np.ndarray:
    """[NSC*2*P, 2*NJ] tiled output -> [S, NJ]."""
    return (
        o_res.reshape(NSC * 2, P, 2, NJ).transpose(0, 2, 1, 3).reshape(S, NJ)
    )


def run(hidden_states: np.ndarray, W_q: np.ndarray, **run_kwargs):
    """Build (cached), run on 8 cores, gather.  Returns (output, results)."""
    if "nc" not in _CACHE:
        _CACHE["nc"] = build_kernel()
    nc = _CACHE["nc"]
    in_maps = make_in_maps(hidden_states, W_q)
    res = run_bass_kernel_spmd(nc, in_maps, list(range(N_CORES)), **run_kwargs)
    B = N_CORES // 2
    out = np.empty((B, S, H), dtype=np.float32)
    for c in range(N_CORES):
        b, j = c // 2, c % 2
        out[b, :, j * NJ : (j + 1) * NJ] = untile_out(res.results[c]["o"])
    return out, res


def kernel(hidden_states: np.ndarray, W_q: np.ndarray, **unused) -> np.ndarray:
    out, _ = run(hidden_states, W_q)
    return out


if __name__ == "__main__":
    rng = np.random.default_rng(0)
    x = rng.standard_normal((4, S, H), dtype=np.float32)
    w = (rng.standard_normal((H, H), dtype=np.float32) * 9.02e-5).astype(np.float32)
    out = kernel(hidden_states=x, W_q=w)
    xb = x[0].astype(np.float64)
    ref0 = xb @ w.astype(np.float64).T @ (xb.T @ xb)
    err = np.abs(out[0] - ref0) / (np.abs(ref0).max() + 1e-30)
    print("max scale-relative err (batch 0):", err.max())
